# revision 33
# baseline (speedup 1.0000x reference)
"""CapsuleLayer (dynamic routing) Trainium2 Bass kernel, v2.

Problem: u_hat = einsum('bi,crio->bcro', x, W); 3 iterations of dynamic
routing (softmax over capsule dim C, squash over OUT dim) -> v (B, R, OUT).

  B=64, C=32, R=1152, IN=128, OUT=16, ITERS=3.

Sharding: routes R across the 8 cores (144 each); softmax is over C which
stays local, so no collectives and each core reads 1/8 of the weights.

v2 changes vs the 483us baseline:
  - W host-pretransposed to (IN, r, c, o): DMA lands the contraction dim IN
    on partitions, eliminating all 576 per-core PE transposes.
  - u_hat matmuls are 3 chained fp16 passes (x_hi@W_hi + x_lo@W_hi +
    x_hi@W_lo) at 1 cycle/column instead of fp32's 4.  Host prescales x and
    W by 2^10 so the fp16 lo-parts stay normal; the PSUM evac folds the
    2^-20 descale.  u_hat accuracy ~2^-20 rel (routing needs ~2^-18; plain
    bf16/fp16/fp22 matmuls all fail the 2e-2 gate).
  - iteration-1's sum_c u_hat comes from a small x @ (sum_c W) matmul on the
    PE (host-precomputed W-sum) instead of a DVE reduction pass.
  - routing elementwise work splits across DVE and GPSIMD: Pool runs 3 of
    the 4 big mul passes per chunk-part, DVE runs the 4th plus all
    reductions and small ops.
"""

import functools
import os

import numpy as np

B, C, R, IN, OUT = 64, 32, 1152, 128, 16
ITERS = 3
NCORES = 8
RL = R // NCORES            # routes per core = 144
RS = RL // 2                # route-slot pairs per core = 72
NCH = 6                     # chunks
RC = RL // NCH              # routes per chunk = 36
PH = RC // 4                # rs-slots per routing part = 6
NCO = RL * C * OUT          # wh/wl u_hat columns per core = 73728
NCOT = NCO + RL * OUT       # + wsum columns appended = 76032
XS = 10                     # host prescale 2^XS on x and W fp16 halves
DESCALE = float(2.0 ** (-2 * XS))


def _bcast(ap, dim_idx, count):
    """Insert a broadcast (stride-0) dim at dim_idx (free dims are 1-based
    after the partition dim)."""
    import concourse.bass as bass

    dims = [list(d) for d in ap.ap]
    dims.insert(dim_idx, [0, count])
    return bass.AP(tensor=ap.tensor, offset=ap.offset, ap=dims)


def _ap(tensor_ap, offset_elems, dims):
    import concourse.bass as bass

    return bass.AP(
        tensor=tensor_ap.tensor, offset=tensor_ap.offset + offset_elems, ap=dims
    )


@functools.lru_cache(maxsize=2)
def _build(debug=False):
    import concourse.bacc as bacc
    import concourse.tile as tile
    from concourse import mybir
    from concourse.masks import make_identity

    f32 = mybir.dt.float32
    f16 = mybir.dt.float16
    AX = mybir.AxisListType
    OP = mybir.AluOpType
    AF = mybir.ActivationFunctionType

    nc = bacc.Bacc(None, target_bir_lowering=False, debug=False)

    wh = nc.dram_tensor("wh", [IN, NCOT], f16, kind="ExternalInput")
    wl = nc.dram_tensor("wl", [IN, NCOT], f16, kind="ExternalInput")
    x = nc.dram_tensor("x", [B, IN], f32, kind="ExternalInput")
    vout = nc.dram_tensor("v", [B, RL, OUT], f32, kind="ExternalOutput")
    dbg = (
        nc.dram_tensor("dbg", [128, C, RS, OUT], f32, kind="ExternalOutput")
        if debug
        else None
    )

    with tile.TileContext(nc) as tc:
        with (
            tc.tile_pool(name="consts", bufs=1) as consts,
            tc.tile_pool(name="whp", bufs=2) as whp,
            tc.tile_pool(name="wlp", bufs=2) as wlp,
            tc.tile_pool(name="u", bufs=3) as u_pool,
            tc.tile_pool(name="sm", bufs=3) as sm_pool,
            tc.tile_pool(name="tmpg", bufs=2) as tmpg_pool,
            tc.tile_pool(name="tmps", bufs=1) as tmps_pool,
            tc.tile_pool(name="psu", bufs=2, space="PSUM") as psum_u,
        ):
            ident = consts.tile([128, 128], f32)
            make_identity(nc, ident)

            # Single ACT table set with Copy/Identity/Square/Ln/Exp.
            from concourse.hw_specs import get_activation_tables

            _tabs = list(get_activation_tables(nc.m.arch))
            _set_id = _tabs.index("natural_log_exp_and_others")
            nc.scalar.add_instruction(
                mybir.InstLoadActFuncSet(
                    name=nc.get_next_instruction_name(),
                    ins=[],
                    outs=[],
                    act_func_set_id=_set_id,
                )
            )

            # ---------------- x prep ----------------
            x_sb = consts.tile([B, IN], f32)
            nc.sync.dma_start(out=x_sb[:], in_=x[:])
            xT_ps = psum_u.tile([128, 4, 512], f32, tag="up")
            nc.tensor.transpose(xT_ps[:, 0, 0:B], x_sb[:], ident[0:B, 0:B])
            # scaled fp16 hi/lo stationaries [xh|xh], [xl|xl]
            xs32 = consts.tile([128, B], f32)
            nc.scalar.activation(
                xs32[:], xT_ps[:, 0, 0:B], AF.Copy, scale=float(2.0**XS)
            )
            xh16 = consts.tile([128, 2, B], f16)
            nc.scalar.activation(xh16[:, 0, :], xs32[:], AF.Copy)
            nc.scalar.activation(xh16[:, 1, :], xs32[:], AF.Copy)
            xl32 = consts.tile([128, B], f32)
            nc.vector.tensor_sub(xl32[:], xs32[:], xh16[:, 0, :])
            xl16 = consts.tile([128, 2, B], f16)
            nc.scalar.activation(xl16[:, 0, :], xl32[:], AF.Copy)
            nc.scalar.activation(xl16[:, 1, :], xl32[:], AF.Copy)
            xh16f = xh16.rearrange("p d b -> p (d b)")
            xl16f = xl16.rearrange("p d b -> p (d b)")

            # ---------------- iter-1 v1 from wsum ----------------
            # wsum rides as fp16 hi/lo columns [NCO, NCOT) of wh/wl.
            # S1[b, rs, o] for parity par lives in partition half par.
            ws_h = consts.tile([128, RL * OUT], f16)
            ws_l = consts.tile([128, RL * OUT], f16)
            nc.sync.dma_start(out=ws_h[:], in_=wh[:, NCO:NCOT])
            nc.sync.dma_start(out=ws_l[:], in_=wl[:, NCO:NCOT])
            ws_hf = ws_h
            ws_lf = ws_l
            S1sb = consts.tile([128, RS, OUT], f32)
            for par in range(2):
                ps1 = psum_u.tile([128, 4, 512], f32, tag="up")
                for blk in range(3):  # 512, 512, 128 columns
                    ncols = (512, 512, 128)[blk]
                    nrs = ncols // OUT
                    rs0 = blk * 32
                    off = (2 * rs0 + par) * OUT
                    src_h = _ap(
                        ws_hf[:], off, [list(ws_hf[:].ap[0]), [2 * OUT, nrs], [1, OUT]]
                    )
                    src_l = _ap(
                        ws_lf[:], off, [list(ws_lf[:].ap[0]), [2 * OUT, nrs], [1, OUT]]
                    )
                    nc.tensor.matmul(
                        ps1[:, blk, 0:ncols], xh16f[:], src_h, start=True, stop=False
                    )
                    nc.tensor.matmul(
                        ps1[:, blk, 0:ncols], xl16f[:], src_h, start=False, stop=False
                    )
                    nc.tensor.matmul(
                        ps1[:, blk, 0:ncols], xh16f[:], src_l, start=False, stop=True
                    )
                nc.scalar.activation(
                    S1sb[64 * par : 64 * par + 64, :, :],
                    ps1.rearrange("p s n -> p (s n)")[
                        64 * par : 64 * par + 64, 0 : RS * OUT
                    ].rearrange("p (r o) -> p r o", o=OUT),
                    AF.Copy,
                    scale=DESCALE / C,
                )

            # v1 = squash(S1): w = n2 / ((1+n2)*n0), n0 = sqrt(n2).
            # Squares go through v1's own tile (overwritten at the end).
            v1 = consts.tile([128, RS, OUT], f32)
            nc.scalar.activation(v1[:], S1sb[:], AF.Square)
            n2_1 = consts.tile([128, RS], f32)
            nc.vector.tensor_reduce(n2_1[:], v1[:], axis=AX.X, op=OP.add)
            n0_1 = consts.tile([128, RS], f32)
            nc.scalar.activation(n0_1[:], n2_1[:], AF.Ln)
            nc.scalar.activation(n0_1[:], n0_1[:], AF.Exp, scale=0.5)
            t1_1 = consts.tile([128, RS], f32)
            nc.scalar.add(t1_1[:], n2_1[:], 1.0)
            nc.vector.tensor_mul(t1_1[:], t1_1[:], n0_1[:])
            nc.vector.reciprocal(t1_1[:], t1_1[:])
            w1 = consts.tile([128, RS], f32)
            nc.vector.tensor_mul(w1[:], n2_1[:], t1_1[:])
            nc.vector.tensor_mul(v1[:], S1sb[:], _bcast(w1[:], 2, OUT))

            # ---------------- chunk loop ----------------
            # Chain emission is interleaved across parts AND chunks so each
            # engine's static order holds ready work from ~4 chains at once.
            live_chains = []

            def step_chains(n):
                for _ in range(n):
                    for g in list(live_chains):
                        try:
                            next(g)
                        except StopIteration:
                            live_chains.remove(g)

            for ch in range(NCH):
                rb = ch * RC
                u = u_pool.tile([128, C, RC // 2, OUT], f32, tag="u", name="u")

                whts, wlts = [], []
                for grp in range(RC // 4):  # 4-route DMA groups
                    gr = rb + grp * 4
                    wht = whp.tile([128, 4, 512], f16, tag="wht")
                    wlt = wlp.tile([128, 4, 512], f16, tag="wlt")
                    nc.sync.dma_start(
                        out=wht[:],
                        in_=wh[:, gr * 512 : (gr + 4) * 512].rearrange(
                            "p (r n) -> p r n", n=512
                        ),
                    )
                    nc.sync.dma_start(
                        out=wlt[:],
                        in_=wl[:, gr * 512 : (gr + 4) * 512].rearrange(
                            "p (r n) -> p r n", n=512
                        ),
                    )
                    whts.append(wht)
                    wlts.append(wlt)
                for pt in range(RC // 4):  # 4 routes per psum tile
                    pp = psum_u.tile([128, 4, 512], f32, tag="up")
                    for rg in range(4):
                        r = 4 * pt + rg  # route within chunk
                        nc.tensor.matmul(
                            pp[:, rg, :], xh16f[:], whts[r // 4][:, r % 4, :],
                            start=True, stop=False,
                        )
                        nc.tensor.matmul(
                            pp[:, rg, :], xl16f[:], whts[r // 4][:, r % 4, :],
                            start=False, stop=False,
                        )
                        nc.tensor.matmul(
                            pp[:, rg, :], xh16f[:], wlts[r // 4][:, r % 4, :],
                            start=False, stop=True,
                        )
                    # evac 4 routes -> u[:, :, rs0:rs0+2, :] with descale.
                    # slot = 2*rs2 + par; half par takes partitions par*64..
                    # and slots {par, par+2}.
                    rs0 = pt * 2
                    ppv = pp.rearrange("p (s2 par) n -> p par s2 n", par=2)
                    with tc.high_priority(offset=60):
                        for h in range(2):
                            src = ppv[64 * h : 64 * h + 64, h].rearrange(
                                "p s (c o) -> p c s o", o=OUT
                            )
                            nc.scalar.activation(
                                u[64 * h : 64 * h + 64, :, rs0 : rs0 + 2, :],
                                src,
                                AF.Copy,
                                scale=DESCALE,
                            )

                if dbg is not None:
                    nc.sync.dma_start(
                        out=dbg[:, :, ch * (RC // 2) : (ch + 1) * (RC // 2), :],
                        in_=u[:],
                    )

                # ---------------- routing ----------------
                def chain(part, ch=ch, u=u):
                    pg = f"{ch}{part}"
                    rsl = slice(part * PH, (part + 1) * PH)
                    up_ = u[:, :, rsl, :]  # (128, C, PH, OUT)
                    rs_g = ch * (RC // 2) + part * PH
                    v1p = v1[:, rs_g : rs_g + PH, :]

                    def stile(shape, tag):
                        return sm_pool.tile(
                            shape, f32, tag=tag + f"{part}", name=tag + pg
                        )

                    def squash_stages(S, rz, tag):
                        # v = w * (S*rz), w = sqrt(n2)/(1+n2), n2 = |S*rz|^2.
                        # sqrt via exp(0.5*ln): ~1e-6 rel, no Newton refine.
                        sq = stile([128, PH, OUT], "sq")
                        nc.scalar.activation(sq[:], S[:], AF.Square)
                        n2 = stile([128, PH], "n2" + tag)
                        nc.vector.tensor_reduce(n2[:], sq[:], axis=AX.X, op=OP.add)
                        if rz is not None:
                            zq = stile([128, PH], "zq")
                            nc.vector.tensor_mul(zq[:], rz[:], rz[:])
                            nc.vector.tensor_mul(n2[:], n2[:], zq[:])
                        yield
                        n0 = stile([128, PH], "n0")
                        nc.scalar.activation(n0[:], n2[:], AF.Ln)
                        nc.scalar.activation(n0[:], n0[:], AF.Exp, scale=0.5)
                        t1 = stile([128, PH], "t1")
                        nc.scalar.add(t1[:], n2[:], 1.0)
                        nc.vector.reciprocal(t1[:], t1[:])
                        yield
                        wsc = stile([128, PH], "wsc")
                        nc.vector.tensor_mul(wsc[:], n0[:], t1[:])
                        if rz is not None:
                            nc.vector.tensor_mul(wsc[:], wsc[:], rz[:])
                        v = stile([128, PH, OUT], "v" + tag)
                        nc.vector.tensor_mul(v[:], S[:], _bcast(wsc[:], 2, OUT))
                        squash_stages.result = v

                    def softmax_stages(blog):
                        m = stile([128, PH], "m")
                        nc.vector.tensor_reduce(
                            m[:],
                            blog.rearrange("p c r -> p r c"),
                            axis=AX.X,
                            op=OP.max,
                        )
                        e = stile([128, C, PH], "e")
                        nc.vector.tensor_sub(e[:], blog[:], _bcast(m[:], 1, C))
                        nc.scalar.activation(e[:], e[:], AF.Exp)
                        yield
                        rz = stile([128, PH], "z")
                        nc.vector.tensor_reduce(
                            rz[:],
                            e.rearrange("p c r -> p r c"),
                            axis=AX.X,
                            op=OP.add,
                        )
                        nc.vector.reciprocal(rz[:], rz[:])
                        softmax_stages.result = (e, rz)

                    # ---- iter 2: blog2 = u . v1 (mul Pool, o-red DVE) ----
                    tt1 = tmpg_pool.tile([128, C, PH, OUT], f32, tag=f"g{part}", name="tt1" + pg)
                    nc.gpsimd.tensor_mul(tt1[:], up_, _bcast(v1p, 1, C))
                    yield
                    blog = stile([128, C, PH], "blog")
                    nc.vector.tensor_reduce(blog[:], tt1[:], axis=AX.X, op=OP.add)
                    yield
                    yield from softmax_stages(blog)
                    e2, rz2 = softmax_stages.result
                    yield

                    # ---- S2 = sum_c e2*u (mul Pool, c-red DVE) ----
                    tt2 = tmps_pool.tile([128, C, PH, OUT], f32, tag=f"s{part}", name="tt2" + pg)
                    nc.gpsimd.tensor_mul(tt2[:], up_, _bcast(e2[:], 3, OUT))
                    yield
                    S2 = stile([128, PH, OUT], "S2")
                    nc.vector.tensor_reduce(
                        S2[:], tt2.rearrange("p c r o -> p r o c"), axis=AX.X, op=OP.add
                    )
                    yield
                    yield from squash_stages(S2, rz2, "2")
                    v2 = squash_stages.result
                    yield

                    # ---- blog3 += u . v2 (mul Pool, o-red DVE) ----
                    tt3 = tmpg_pool.tile([128, C, PH, OUT], f32, tag=f"g{part}", name="tt3" + pg)
                    nc.gpsimd.tensor_mul(tt3[:], up_, _bcast(v2[:], 1, C))
                    yield
                    g2 = stile([128, C, PH], "g2")
                    nc.vector.tensor_reduce(g2[:], tt3[:], axis=AX.X, op=OP.add)
                    nc.vector.tensor_add(blog[:], blog[:], g2[:])
                    yield
                    yield from softmax_stages(blog)
                    e3, rz3 = softmax_stages.result
                    yield

                    # ---- S3 = sum_c e3*u (mul DVE in-place into u, c-red DVE) ----
                    nc.vector.tensor_mul(up_, up_, _bcast(e3[:], 3, OUT))
                    yield
                    S3 = stile([128, PH, OUT], "S3")
                    nc.vector.tensor_reduce(
                        S3[:],
                        u[:, :, rsl, :].rearrange("p c r o -> p r o c"),
                        axis=AX.X,
                        op=OP.add,
                    )
                    yield
                    yield from squash_stages(S3, rz3, "3")
                    v3 = squash_stages.result

                    for rhat in range(2):
                        nc.sync.dma_start(
                            out=_ap(
                                vout[:],
                                (2 * rs_g + rhat) * OUT,
                                [[RL * OUT, B], [2 * OUT, PH], [1, OUT]],
                            ),
                            in_=v3[64 * rhat : 64 * rhat + 64, :, :],
                        )

                live_chains.append(chain(0))
                live_chains.append(chain(1))
                if ch < NCH - 1:
                    step_chains(7)
            step_chains(10**6)

    nc.compile()
    return nc


def kernel(x: np.ndarray, route_weights: np.ndarray) -> np.ndarray:
    from concourse.bass_utils import run_bass_kernel_spmd

    debug = bool(int(os.environ.get("CAPS_DEBUG", "0")))
    nc = _build(debug)

    xh = np.ascontiguousarray(np.asarray(x, dtype=np.float32).reshape(B, IN))
    W = np.asarray(route_weights, dtype=np.float32)

    in_maps = []
    for k in range(NCORES):
        wk = W[:, k * RL : (k + 1) * RL]          # (C, RL, IN, OUT)
        wk = wk.transpose(2, 1, 0, 3)              # (IN, RL, C, OUT)
        ws64 = wk.astype(np.float64) * float(2.0**XS)
        sum64 = ws64.sum(axis=2)                   # (IN, RL, OUT) scaled wsum
        whk = np.empty((IN, NCOT), np.float16)
        wlk = np.empty((IN, NCOT), np.float16)
        whk[:, :NCO] = ws64.reshape(IN, NCO)
        wlk[:, :NCO] = (
            ws64 - whk[:, :NCO].reshape(IN, RL, C, OUT).astype(np.float64)
        ).reshape(IN, NCO)
        whk[:, NCO:] = sum64.reshape(IN, RL * OUT)
        wlk[:, NCO:] = (
            sum64 - whk[:, NCO:].reshape(IN, RL, OUT).astype(np.float64)
        ).reshape(IN, RL * OUT)
        in_maps.append({"wh": whk, "wl": wlk, "x": xh})

    res = run_bass_kernel_spmd(
        nc,
        in_maps,
        core_ids=list(range(NCORES)),
        trace=False,
    )
    out = np.concatenate([r["v"] for r in res.results], axis=1)
    if debug:
        kernel.last_dbg = [r["dbg"] for r in res.results]  # type: ignore[attr-defined]
    return out


# revision 46
# speedup vs baseline: 1.0682x; 1.0682x over previous
"""CapsuleLayer (dynamic routing) Trainium2 Bass kernel, v2.

Problem: u_hat = einsum('bi,crio->bcro', x, W); 3 iterations of dynamic
routing (softmax over capsule dim C, squash over OUT dim) -> v (B, R, OUT).

  B=64, C=32, R=1152, IN=128, OUT=16, ITERS=3.

Sharding: routes R across the 8 cores (144 each); softmax is over C which
stays local, so no collectives and each core reads 1/8 of the weights.

v2 changes vs the 483us baseline:
  - W host-pretransposed to (IN, r, c, o): DMA lands the contraction dim IN
    on partitions, eliminating all 576 per-core PE transposes.
  - u_hat matmuls are 3 chained fp16 passes (x_hi@W_hi + x_lo@W_hi +
    x_hi@W_lo) at 1 cycle/column instead of fp32's 4.  Host prescales x and
    W by 2^10 so the fp16 lo-parts stay normal; the PSUM evac folds the
    2^-20 descale.  u_hat accuracy ~2^-20 rel (routing needs ~2^-18; plain
    bf16/fp16/fp22 matmuls all fail the 2e-2 gate).
  - iteration-1's sum_c u_hat comes from a small x @ (sum_c W) matmul on the
    PE (host-precomputed W-sum) instead of a DVE reduction pass.
  - routing elementwise work splits across DVE and GPSIMD: Pool runs 3 of
    the 4 big mul passes per chunk-part, DVE runs the 4th plus all
    reductions and small ops.
"""

import functools
import os

import numpy as np

B, C, R, IN, OUT = 64, 32, 1152, 128, 16
ITERS = 3
NCORES = 8
RL = R // NCORES            # routes per core = 144
RS = RL // 2                # route-slot pairs per core = 72
NCH = 6                     # chunks
RC = RL // NCH              # routes per chunk = 36
PH = RC // 4                # rs-slots per routing part = 6
NCO = RL * C * OUT          # wh/wl u_hat columns per core = 73728
NCOT = NCO + RL * OUT       # + wsum columns appended = 76032
XS = 10                     # host prescale 2^XS on x and W fp16 halves
DESCALE = float(2.0 ** (-2 * XS))


def _bcast(ap, dim_idx, count):
    """Insert a broadcast (stride-0) dim at dim_idx (free dims are 1-based
    after the partition dim)."""
    import concourse.bass as bass

    dims = [list(d) for d in ap.ap]
    dims.insert(dim_idx, [0, count])
    return bass.AP(tensor=ap.tensor, offset=ap.offset, ap=dims)


def _ap(tensor_ap, offset_elems, dims):
    import concourse.bass as bass

    return bass.AP(
        tensor=tensor_ap.tensor, offset=tensor_ap.offset + offset_elems, ap=dims
    )


@functools.lru_cache(maxsize=2)
def _build(debug=False):
    import concourse.bacc as bacc
    import concourse.tile as tile
    from concourse import mybir
    from concourse.masks import make_identity

    f32 = mybir.dt.float32
    f16 = mybir.dt.float16
    AX = mybir.AxisListType
    OP = mybir.AluOpType
    AF = mybir.ActivationFunctionType

    nc = bacc.Bacc(None, target_bir_lowering=False, debug=False)

    wh = nc.dram_tensor("wh", [IN, NCOT], f16, kind="ExternalInput")
    wl = nc.dram_tensor("wl", [IN, NCOT], f16, kind="ExternalInput")
    x = nc.dram_tensor("x", [B, IN], f32, kind="ExternalInput")
    vout = nc.dram_tensor("v", [B, RL, OUT], f32, kind="ExternalOutput")
    dbg = (
        nc.dram_tensor("dbg", [128, C, RS, OUT], f32, kind="ExternalOutput")
        if debug
        else None
    )

    with tile.TileContext(nc) as tc:
        with (
            tc.tile_pool(name="consts", bufs=1) as consts,
            tc.tile_pool(name="whp", bufs=2) as whp,
            tc.tile_pool(name="wlp", bufs=2) as wlp,
            tc.tile_pool(name="u", bufs=3) as u_pool,
            tc.tile_pool(name="sm", bufs=3) as sm_pool,
            tc.tile_pool(name="tmpg", bufs=2) as tmpg_pool,
            tc.tile_pool(name="tmps", bufs=1) as tmps_pool,
            tc.tile_pool(name="psu", bufs=2, space="PSUM") as psum_u,
        ):
            ident = consts.tile([128, 128], f32)
            make_identity(nc, ident)

            # Single ACT table set with Copy/Identity/Square/Ln/Exp.
            from concourse.hw_specs import get_activation_tables

            _tabs = list(get_activation_tables(nc.m.arch))
            _set_id = _tabs.index("natural_log_exp_and_others")
            nc.scalar.add_instruction(
                mybir.InstLoadActFuncSet(
                    name=nc.get_next_instruction_name(),
                    ins=[],
                    outs=[],
                    act_func_set_id=_set_id,
                )
            )

            # ---------------- x prep ----------------
            x_sb = consts.tile([B, IN], f32)
            nc.sync.dma_start(out=x_sb[:], in_=x[:])
            xT_ps = psum_u.tile([128, 4, 512], f32, tag="up")
            nc.tensor.transpose(xT_ps[:, 0, 0:B], x_sb[:], ident[0:B, 0:B])
            # scaled fp16 hi/lo stationaries [xh|xh], [xl|xl]
            xs32 = consts.tile([128, B], f32)
            nc.scalar.activation(
                xs32[:], xT_ps[:, 0, 0:B], AF.Copy, scale=float(2.0**XS)
            )
            xh16 = consts.tile([128, 2, B], f16)
            nc.scalar.activation(xh16[:, 0, :], xs32[:], AF.Copy)
            nc.scalar.activation(xh16[:, 1, :], xs32[:], AF.Copy)
            xl32 = consts.tile([128, B], f32)
            nc.vector.tensor_sub(xl32[:], xs32[:], xh16[:, 0, :])
            xl16 = consts.tile([128, 2, B], f16)
            nc.scalar.activation(xl16[:, 0, :], xl32[:], AF.Copy)
            nc.scalar.activation(xl16[:, 1, :], xl32[:], AF.Copy)
            xh16f = xh16.rearrange("p d b -> p (d b)")
            xl16f = xl16.rearrange("p d b -> p (d b)")

            # ---------------- iter-1 v1 from wsum ----------------
            # wsum rides as fp16 hi/lo columns [NCO, NCOT) of wh/wl.
            # S1[b, rs, o] for parity par lives in partition half par.
            ws_h = consts.tile([128, RL * OUT], f16)
            ws_l = consts.tile([128, RL * OUT], f16)
            nc.sync.dma_start(out=ws_h[:], in_=wh[:, NCO:NCOT])
            nc.sync.dma_start(out=ws_l[:], in_=wl[:, NCO:NCOT])
            ws_hf = ws_h
            ws_lf = ws_l
            S1sb = consts.tile([128, RS, OUT], f32)
            for par in range(2):
                ps1 = psum_u.tile([128, 4, 512], f32, tag="up")
                for blk in range(3):  # 512, 512, 128 columns
                    ncols = (512, 512, 128)[blk]
                    nrs = ncols // OUT
                    rs0 = blk * 32
                    off = (2 * rs0 + par) * OUT
                    src_h = _ap(
                        ws_hf[:], off, [list(ws_hf[:].ap[0]), [2 * OUT, nrs], [1, OUT]]
                    )
                    src_l = _ap(
                        ws_lf[:], off, [list(ws_lf[:].ap[0]), [2 * OUT, nrs], [1, OUT]]
                    )
                    nc.tensor.matmul(
                        ps1[:, blk, 0:ncols], xh16f[:], src_h, start=True, stop=False
                    )
                    nc.tensor.matmul(
                        ps1[:, blk, 0:ncols], xl16f[:], src_h, start=False, stop=False
                    )
                    nc.tensor.matmul(
                        ps1[:, blk, 0:ncols], xh16f[:], src_l, start=False, stop=True
                    )
                nc.scalar.activation(
                    S1sb[64 * par : 64 * par + 64, :, :],
                    ps1.rearrange("p s n -> p (s n)")[
                        64 * par : 64 * par + 64, 0 : RS * OUT
                    ].rearrange("p (r o) -> p r o", o=OUT),
                    AF.Copy,
                    scale=DESCALE / C,
                )

            # v1 = squash(S1): w = n2 / ((1+n2)*n0), n0 = sqrt(n2).
            # Squares go through v1's own tile (overwritten at the end).
            v1 = consts.tile([128, RS, OUT], f32)
            nc.scalar.activation(v1[:], S1sb[:], AF.Square)
            n2_1 = consts.tile([128, RS], f32)
            nc.vector.tensor_reduce(n2_1[:], v1[:], axis=AX.X, op=OP.add)
            n0_1 = consts.tile([128, RS], f32)
            nc.scalar.activation(n0_1[:], n2_1[:], AF.Ln)
            nc.scalar.activation(n0_1[:], n0_1[:], AF.Exp, scale=0.5)
            t1_1 = consts.tile([128, RS], f32)
            nc.scalar.add(t1_1[:], n2_1[:], 1.0)
            nc.vector.tensor_mul(t1_1[:], t1_1[:], n0_1[:])
            nc.vector.reciprocal(t1_1[:], t1_1[:])
            w1 = consts.tile([128, RS], f32)
            nc.vector.tensor_mul(w1[:], n2_1[:], t1_1[:])
            nc.vector.tensor_mul(v1[:], S1sb[:], _bcast(w1[:], 2, OUT))

            # ---------------- chunk loop ----------------
            # Chain emission is interleaved across parts AND chunks so each
            # engine's static order holds ready work from ~4 chains at once.
            live_chains = []

            def step_chains(n):
                for _ in range(n):
                    for g in list(live_chains):
                        try:
                            next(g)
                        except StopIteration:
                            live_chains.remove(g)

            CHUNK_SIZES = [12, 24, 24, 24, 24, 24, 12]
            starts = [sum(CHUNK_SIZES[:i]) for i in range(len(CHUNK_SIZES))]
            for ch in range(len(CHUNK_SIZES)):
                rb = starts[ch]
                rc = CHUNK_SIZES[ch]
                u = u_pool.tile([128, C, RC // 2, OUT], f32, tag="u", name="u")

                whts, wlts = [], []
                for grp in range(rc // 4):  # 4-route DMA groups
                    gr = rb + grp * 4
                    wht = whp.tile([128, 4, 512], f16, tag="wht")
                    wlt = wlp.tile([128, 4, 512], f16, tag="wlt")
                    nc.sync.dma_start(
                        out=wht[:],
                        in_=wh[:, gr * 512 : (gr + 4) * 512].rearrange(
                            "p (r n) -> p r n", n=512
                        ),
                    )
                    nc.sync.dma_start(
                        out=wlt[:],
                        in_=wl[:, gr * 512 : (gr + 4) * 512].rearrange(
                            "p (r n) -> p r n", n=512
                        ),
                    )
                    whts.append(wht)
                    wlts.append(wlt)
                for pt in range(rc // 4):  # 4 routes per psum tile
                    pp = psum_u.tile([128, 4, 512], f32, tag="up")
                    for rg in range(4):
                        r = 4 * pt + rg  # route within chunk
                        nc.tensor.matmul(
                            pp[:, rg, :], xh16f[:], whts[r // 4][:, r % 4, :],
                            start=True, stop=False,
                        )
                        nc.tensor.matmul(
                            pp[:, rg, :], xl16f[:], whts[r // 4][:, r % 4, :],
                            start=False, stop=False,
                        )
                        nc.tensor.matmul(
                            pp[:, rg, :], xh16f[:], wlts[r // 4][:, r % 4, :],
                            start=False, stop=True,
                        )
                    # evac 4 routes -> u[:, :, rs0:rs0+2, :] with descale.
                    # slot = 2*rs2 + par; half par takes partitions par*64..
                    # and slots {par, par+2}.
                    rs0 = pt * 2
                    ppv = pp.rearrange("p (s2 par) n -> p par s2 n", par=2)
                    with tc.high_priority(offset=150):
                        for h in range(2):
                            src = ppv[64 * h : 64 * h + 64, h].rearrange(
                                "p s (c o) -> p c s o", o=OUT
                            )
                            nc.scalar.activation(
                                u[64 * h : 64 * h + 64, :, rs0 : rs0 + 2, :],
                                src,
                                AF.Copy,
                                scale=DESCALE,
                            )

                if dbg is not None:
                    nc.sync.dma_start(
                        out=dbg[:, :, rb // 2 : rb // 2 + rc // 2, :],
                        in_=u[:, :, 0 : rc // 2, :],
                    )

                # ---------------- routing ----------------
                def chain(part, ch=ch, u=u, rb=rb, rc=rc):
                    phc = rc // 4
                    pg = f"{ch}{part}"
                    rsl = slice(part * phc, (part + 1) * phc)
                    up_ = u[:, :, rsl, :]  # (128, C, phc, OUT)
                    rs_g = rb // 2 + part * phc
                    v1p = v1[:, rs_g : rs_g + phc, :]

                    def stile(shape, tag):
                        # allocate at PH-max so tag sizes stay uniform,
                        # return a phc-sized view
                        fshape = [PH if d == phc else d for d in shape]
                        fshape[0] = 128
                        t = sm_pool.tile(fshape, f32, tag=tag + f"{part}", name=tag + pg)
                        if phc == PH:
                            return t
                        idx = tuple(
                            slice(0, phc) if d == phc else slice(None) for d in shape
                        )
                        return t[idx]

                    def squash_stages(S, z, tag):
                        # S here is the UNNORMALIZED sum_c e*u; the softmax
                        # normalizer z cancels: v = squash(S/z) =
                        # S * sqrt(n2) / (z^2 + n2) with n2 = |S|^2.
                        # sqrt via exp(0.5*ln): ~1e-6 rel.
                        sq = stile([128, phc, OUT], "sq")
                        nc.scalar.activation(sq[:], S[:], AF.Square)
                        n2 = stile([128, phc], "n2" + tag)
                        nc.vector.tensor_reduce(n2[:], sq[:], axis=AX.X, op=OP.add)
                        yield
                        n0 = stile([128, phc], "n0")
                        nc.scalar.activation(n0[:], n2[:], AF.Ln)
                        nc.scalar.activation(n0[:], n0[:], AF.Exp, scale=0.5)
                        q = stile([128, phc], "t1")
                        if z is not None:
                            nc.vector.tensor_mul(q[:], z[:], z[:])
                            nc.vector.tensor_add(q[:], q[:], n2[:])
                        else:
                            nc.scalar.add(q[:], n2[:], 1.0)
                        nc.vector.reciprocal(q[:], q[:])
                        yield
                        wsc = stile([128, phc], "wsc")
                        nc.vector.tensor_mul(wsc[:], n0[:], q[:])
                        v = stile([128, phc, OUT], "v" + tag)
                        nc.vector.tensor_mul(v[:], S[:], _bcast(wsc[:], 2, OUT))
                        squash_stages.result = v

                    def softmax_stages(blog):
                        m = stile([128, phc], "m")
                        nc.vector.tensor_reduce(
                            m[:],
                            blog.rearrange("p c r -> p r c"),
                            axis=AX.X,
                            op=OP.max,
                        )
                        e = stile([128, C, phc], "e")
                        nc.vector.tensor_sub(e[:], blog[:], _bcast(m[:], 1, C))
                        nc.scalar.activation(e[:], e[:], AF.Exp)
                        yield
                        z = stile([128, phc], "z")
                        nc.vector.tensor_reduce(
                            z[:],
                            e.rearrange("p c r -> p r c"),
                            axis=AX.X,
                            op=OP.add,
                        )
                        softmax_stages.result = (e, z)

                    # ---- iter 2: blog2 = u . v1 (mul Pool, o-red DVE) ----
                    tt1 = tmpg_pool.tile([128, C, PH, OUT], f32, tag=f"g{part}", name="tt1" + pg)[
                        :, :, 0:phc, :
                    ]
                    nc.gpsimd.tensor_mul(tt1[:], up_, _bcast(v1p, 1, C))
                    yield
                    blog = stile([128, C, phc], "blog")
                    nc.vector.tensor_reduce(blog[:], tt1[:], axis=AX.X, op=OP.add)
                    yield
                    yield from softmax_stages(blog)
                    e2, rz2 = softmax_stages.result
                    yield

                    # ---- S2 = sum_c e2*u (mul Pool, c-red DVE) ----
                    tt2 = tmps_pool.tile([128, C, PH, OUT], f32, tag=f"s{part}", name="tt2" + pg)[
                        :, :, 0:phc, :
                    ]
                    nc.gpsimd.tensor_mul(tt2[:], up_, _bcast(e2[:], 3, OUT))
                    yield
                    S2 = stile([128, phc, OUT], "S2")
                    nc.vector.tensor_reduce(
                        S2[:], tt2.rearrange("p c r o -> p r o c"), axis=AX.X, op=OP.add
                    )
                    yield
                    yield from squash_stages(S2, rz2, "2")
                    v2 = squash_stages.result
                    yield

                    # ---- blog3 += u . v2 (mul Pool, o-red DVE) ----
                    tt3 = tmpg_pool.tile([128, C, PH, OUT], f32, tag=f"g{part}", name="tt3" + pg)[
                        :, :, 0:phc, :
                    ]
                    nc.gpsimd.tensor_mul(tt3[:], up_, _bcast(v2[:], 1, C))
                    yield
                    g2 = stile([128, C, phc], "g2")
                    nc.vector.tensor_reduce(g2[:], tt3[:], axis=AX.X, op=OP.add)
                    nc.vector.tensor_add(blog[:], blog[:], g2[:])
                    yield
                    yield from softmax_stages(blog)
                    e3, rz3 = softmax_stages.result
                    yield

                    # ---- S3 = sum_c e3*u (mul DVE in-place into u, c-red DVE) ----
                    nc.vector.tensor_mul(up_, up_, _bcast(e3[:], 3, OUT))
                    yield
                    S3 = stile([128, phc, OUT], "S3")
                    nc.vector.tensor_reduce(
                        S3[:],
                        u[:, :, rsl, :].rearrange("p c r o -> p r o c"),
                        axis=AX.X,
                        op=OP.add,
                    )
                    yield
                    yield from squash_stages(S3, rz3, "3")
                    v3 = squash_stages.result

                    for rhat in range(2):
                        nc.sync.dma_start(
                            out=_ap(
                                vout[:],
                                (2 * rs_g + rhat) * OUT,
                                [[RL * OUT, B], [2 * OUT, phc], [1, OUT]],
                            ),
                            in_=v3[64 * rhat : 64 * rhat + 64, :, :],
                        )

                live_chains.append(chain(0))
                step_chains(4)
                live_chains.append(chain(1))
                if ch < NCH - 1:
                    step_chains(4)
            step_chains(10**6)

    nc.compile()
    return nc


def kernel(x: np.ndarray, route_weights: np.ndarray) -> np.ndarray:
    from concourse.bass_utils import run_bass_kernel_spmd

    debug = bool(int(os.environ.get("CAPS_DEBUG", "0")))
    nc = _build(debug)

    xh = np.ascontiguousarray(np.asarray(x, dtype=np.float32).reshape(B, IN))
    W = np.asarray(route_weights, dtype=np.float32)

    in_maps = []
    for k in range(NCORES):
        wk = W[:, k * RL : (k + 1) * RL]          # (C, RL, IN, OUT)
        wk = wk.transpose(2, 1, 0, 3)              # (IN, RL, C, OUT)
        ws64 = wk.astype(np.float64) * float(2.0**XS)
        sum64 = ws64.sum(axis=2)                   # (IN, RL, OUT) scaled wsum
        whk = np.empty((IN, NCOT), np.float16)
        wlk = np.empty((IN, NCOT), np.float16)
        whk[:, :NCO] = ws64.reshape(IN, NCO)
        wlk[:, :NCO] = (
            ws64 - whk[:, :NCO].reshape(IN, RL, C, OUT).astype(np.float64)
        ).reshape(IN, NCO)
        whk[:, NCO:] = sum64.reshape(IN, RL * OUT)
        wlk[:, NCO:] = (
            sum64 - whk[:, NCO:].reshape(IN, RL, OUT).astype(np.float64)
        ).reshape(IN, RL * OUT)
        in_maps.append({"wh": whk, "wl": wlk, "x": xh})

    res = run_bass_kernel_spmd(
        nc,
        in_maps,
        core_ids=list(range(NCORES)),
        trace=False,
    )
    out = np.concatenate([r["v"] for r in res.results], axis=1)
    if debug:
        kernel.last_dbg = [r["dbg"] for r in res.results]  # type: ignore[attr-defined]
    return out
